# revision 1
# baseline (speedup 1.0000x reference)
"""Trainium2 Bass kernel for nn_CrossAttention (B=128, C=1024).

Math (per sample b):
    scores[i,j] = t[i] * v[j]               (rank-1)
    attn        = softmax(scores, axis=j)
    crossed[i]  = sum_j attn[i,j] * v[j]
              = (sum_j exp(t_i v_j) v_j) / (sum_j exp(t_i v_j))   [= N_i / D_i]
    h = v + crossed
    y = LeakyReLU( h @ Wp + bp )            (3x3 conv on 1x1 spatial == center tap
                                             matmul; BN folded into Wp/bp on host)

Device strategy (8 cores, data-parallel over batch, 16 samples/core):
  - layout: j on partitions (8 chunks of 128), i on free dim.
  - E = exp(v_j * t_i) in ONE ScalarE activation per (sample, j-chunk):
        activation(out=E_c, in_=t_bcast, func=Exp, scale=v_chunk_column)
    (per-partition scale fuses the outer product into the exp pass)
  - N_i, D_i via TensorE: lhsT = [v_chunk | ones] (K=128, M=2), rhs = E chunk,
    accumulating over the 8 j-chunks into PSUM. Sample s%4 lands at PSUM
    partition group 32*(s%4) (matmul out base partition must be 0/32/64/96).
  - drain PSUM -> SBUF (DVE copy), rearrange with SBUF->SBUF DMA, then
    crossed = N * reciprocal(D), h = crossed + v on DVE.
  - h transposed via TensorE (identity matmul) to feed the projection matmul
    y = h @ Wp + bp (fp32 data bitcast to float32r for full-rate PE).
  - LeakyReLU = max(y, 0.1*y) on DVE.
"""

import os
import numpy as np

import concourse.bass as bass
import concourse.bacc as bacc
import concourse.tile as tile
from concourse import mybir
from concourse.bass_utils import run_bass_kernel_spmd
from concourse.masks import make_identity

B, C = 128, 1024
N_CORES = 8
SPC = B // N_CORES          # samples per core (16)
NCH = C // 128              # j-chunks of 128 (8)
GRP = 4                     # samples per PSUM accumulation group
HALF = 512                  # matmul moving-dim tile (fp32r full rate >= 256)
BN_EPS = 1e-5
SLOPE = 0.1
F32 = mybir.dt.float32
F32R = mybir.dt.float32r
EXP = mybir.ActivationFunctionType.Exp
COPY = mybir.ActivationFunctionType.Copy

# Number of j-chunks per sample whose S = t*v product is precomputed on the
# VectorE (tensor_scalar, 2x fp32) and exp'd in one big ScalarE pass; the
# remaining chunks fuse the multiply into the activation via per-partition
# scale. 0 = fully fused (ScalarE-heavy), 8 = fully precomputed (DVE-heavy).
DVE_CHUNKS = 0


def build_nc(dve_chunks: int = DVE_CHUNKS, repeat: int = 1, mode: str = "full"):
    nc = bacc.Bacc("TRN2", target_bir_lowering=False, debug=False,
                   num_devices=N_CORES)
    t_d = nc.dram_tensor("t", [SPC, C], F32, kind="ExternalInput").ap()
    v_d = nc.dram_tensor("v", [SPC, C], F32, kind="ExternalInput").ap()
    w_d = nc.dram_tensor("w", [C, C], F32R, kind="ExternalInput").ap()
    b_d = nc.dram_tensor("b", [1, C], F32, kind="ExternalInput").ap()
    o_d = nc.dram_tensor("o", [SPC, C], F32, kind="ExternalOutput").ap()

    with tile.TileContext(nc) as tc:
        for _ in range(repeat):
            _body(nc, tc, t_d, v_d, w_d, b_d, o_d, dve_chunks, mode)
    nc.compile()
    return nc


def _body(nc, tc, t_d, v_d, w_d, b_d, o_d, dve_chunks, mode="full"):
    with (
        tc.tile_pool(name="singles", bufs=1) as singles,
        tc.tile_pool(name="tpool", bufs=3) as tpool,
        tc.tile_pool(name="epool", bufs=2) as epool,
        tc.tile_pool(name="spool", bufs=2) as spool,
        tc.tile_pool(name="stpool", bufs=2) as stpool,
        tc.tile_pool(name="ndpsum", bufs=2, space="PSUM") as ndpsum,
        tc.tile_pool(name="trpsum", bufs=2, space="PSUM") as trpsum,
        tc.tile_pool(name="ppsum", bufs=2, space="PSUM") as ppsum,
    ):
        # ---- constants / persistent tiles ----
        ident = singles.tile([SPC, SPC], F32)
        make_identity(nc, ident)

        wp_sb = singles.tile([128, NCH, C], F32R)
        nc.sync.dma_start(out=wp_sb, in_=w_d.rearrange("(c p) o -> p c o", p=128))
        # bias broadcast to all SPC partitions; added on DVE after the matmul
        bp_sb = singles.tile([SPC, C], F32)
        nc.sync.dma_start(out=bp_sb, in_=b_d.to_broadcast([SPC, C]))
        v_sb = singles.tile([SPC, C], F32)
        nc.sync.dma_start(out=v_sb, in_=v_d)
        # per sample: cols [0:C) = N_i, cols [C:2C) = D_i
        nd_sb = singles.tile([SPC, 2 * C], F32)

        # IV_c: (128, 2*SPC); even col 2s = v[sample s, chunk c] (transposed),
        # odd cols = 1.0.  lhsT for (s, c) = IV_c[:, 2s:2s+2] -> [N; D] rows.
        # vts keeps the exact fp32 v columns for the activation scale.
        ones_sp = singles.tile([128, SPC], F32)
        nc.vector.memset(ones_sp, 1.0)
        ivs, vts = [], []
        for c in range(NCH):
            iv = singles.tile([128, 2 * SPC], F32R, tag=f"iv{c}")
            nc.scalar.activation(
                out=iv[:, 1:2 * SPC:2], in_=ones_sp, func=COPY)
            pt = trpsum.tile([128, SPC], F32, tag="pt")
            nc.tensor.transpose(pt, v_sb[:, c * 128:(c + 1) * 128], ident)
            nc.scalar.activation(out=iv[:, 0:2 * SPC:2], in_=pt, func=COPY)
            vt = singles.tile([128, SPC], F32, tag=f"vt{c}")
            nc.vector.tensor_copy(vt, pt)
            ivs.append(iv)
            vts.append(vt)

        # ---- main loop: exp + N/D accumulation ----
        for s in range(SPC):
            nd = ndpsum.tile([2, 2 * HALF], F32, tag="nd")
            tb = tpool.tile([128, C], F32, tag="tb")
            # 4 partition-range chunks -> 4 DMA queues in parallel; a single
            # 128-partition broadcast DMA serializes ~160us/iter on one queue
            # and was the measured bottleneck (exp/mm ablations both ~180us).
            for q in range(4):
                nc.sync.dma_start(
                    out=tb[32 * q:32 * (q + 1), :],
                    in_=t_d[s:s + 1, :].to_broadcast([32, C]))
            e = epool.tile([128, NCH * C], F32R, tag="e")
            if mode == "mm":
                pass
            elif dve_chunks > 0:
                st = spool.tile([128, dve_chunks * C], F32, tag="st")
                for c in range(dve_chunks):
                    nc.vector.tensor_scalar_mul(
                        st[:, c * C:(c + 1) * C], tb, vts[c][:, s:s + 1])
                nc.scalar.activation(
                    out=e[:, 0:dve_chunks * C], in_=st, func=EXP)
            for c in range(dve_chunks, NCH):
                nc.scalar.activation(
                    out=e[:, c * C:(c + 1) * C], in_=tb, func=EXP,
                    scale=vts[c][:, s:s + 1])
            if mode == "exp":
                continue
            for h in range(2):
                for c in range(NCH):
                    nc.tensor.matmul(
                        out=nd[:, h * HALF:(h + 1) * HALF],
                        lhsT=ivs[c][:, 2 * s:2 * s + 2],
                        rhs=e[:, c * C + h * HALF: c * C + (h + 1) * HALF],
                        start=(c == 0), stop=(c == NCH - 1),
                    )
            # drain: rows {0, 1} hold [N; D] of sample s.
            stage = stpool.tile([2, 2 * HALF], F32, tag="stage")
            nc.vector.tensor_copy(stage, nd)
            # partitions {0, 1} -> one row [N | D] of nd_sb.
            nc.sync.dma_start(
                out=nd_sb[s:s + 1, :], in_=stage)

        if mode == "exp":
            nc.sync.dma_start(out=o_d, in_=v_sb)
            return

        # ---- epilogue: crossed = N/D, h = v + crossed ----
        dinv = singles.tile([SPC, C], F32)
        nc.vector.reciprocal(dinv, nd_sb[:, C:2 * C])
        crossed = singles.tile([SPC, C], F32)
        nc.vector.tensor_mul(crossed, nd_sb[:, 0:C], dinv)
        h_sb = singles.tile([SPC, C], F32)
        nc.vector.tensor_add(h_sb, crossed, v_sb)

        # hT chunks (128, SPC) for the projection matmul
        hts = []
        for c in range(NCH):
            pt2 = trpsum.tile([128, SPC], F32, tag="pt")
            nc.tensor.transpose(pt2, h_sb[:, c * 128:(c + 1) * 128], ident)
            ht = singles.tile([128, SPC], F32R, tag=f"ht{c}")
            nc.scalar.activation(out=ht, in_=pt2, func=COPY)
            hts.append(ht)

        # ---- projection + bias + LeakyReLU ----
        out_sb = singles.tile([SPC, C], F32)
        for hh in range(2):
            pp = ppsum.tile([SPC, HALF], F32, tag="pp")
            for c in range(NCH):
                nc.tensor.matmul(
                    out=pp, lhsT=hts[c],
                    rhs=wp_sb[:, c, hh * HALF:(hh + 1) * HALF],
                    start=(c == 0), stop=(c == NCH - 1))
            yb = singles.tile([SPC, HALF], F32, tag=f"yb{hh}")
            nc.vector.tensor_add(yb, pp, bp_sb[:, hh * HALF:(hh + 1) * HALF])
            tmp = singles.tile([SPC, HALF], F32, tag=f"tmp{hh}")
            nc.vector.tensor_scalar_mul(tmp, yb, SLOPE)
            nc.vector.tensor_max(out_sb[:, hh * HALF:(hh + 1) * HALF], yb, tmp)
        nc.sync.dma_start(out=o_d, in_=out_sb)


_NC_CACHE = None
LAST_RESULTS = None


def _prep_host(visual_feat, tactile_feat, conv_w, conv_b, bn_gamma, bn_beta,
               bn_mean, bn_var):
    visual = np.ascontiguousarray(np.asarray(visual_feat, dtype=np.float32))
    tactile = np.ascontiguousarray(np.asarray(tactile_feat, dtype=np.float32))
    conv_w = np.asarray(conv_w, dtype=np.float32)
    conv_b = np.asarray(conv_b, dtype=np.float32)
    gamma = np.asarray(bn_gamma, dtype=np.float32)
    beta = np.asarray(bn_beta, dtype=np.float32)
    mean = np.asarray(bn_mean, dtype=np.float32)
    var = np.asarray(bn_var, dtype=np.float32)

    inv = gamma / np.sqrt(var + np.float32(BN_EPS))        # (C,)
    wc = conv_w[:, :, 1, 1]                                # (O, I) center tap
    wp = np.ascontiguousarray((wc * inv[:, None]).T.astype(np.float32))  # (I, O)
    bp = np.ascontiguousarray(
        (((conv_b - mean) * inv) + beta).astype(np.float32).reshape(1, C))
    return visual, tactile, wp, bp


def kernel(visual_feat, tactile_feat, conv_w, conv_b, bn_gamma, bn_beta,
           bn_mean, bn_var):
    global _NC_CACHE, LAST_RESULTS
    visual, tactile, wp, bp = _prep_host(
        visual_feat, tactile_feat, conv_w, conv_b, bn_gamma, bn_beta,
        bn_mean, bn_var)

    if _NC_CACHE is None:
        _NC_CACHE = build_nc()

    in_maps = []
    for k in range(N_CORES):
        sl = slice(k * SPC, (k + 1) * SPC)
        in_maps.append({
            "t": np.ascontiguousarray(tactile[sl]),
            "v": np.ascontiguousarray(visual[sl]),
            "w": wp,
            "b": bp,
        })
    res = run_bass_kernel_spmd(
        _NC_CACHE, in_maps, core_ids=list(range(N_CORES)),
        trace=bool(int(os.environ.get("KERNEL_TRACE", "0") or "0")),
    )
    LAST_RESULTS = res
    out = np.concatenate([res.results[k]["o"] for k in range(N_CORES)], axis=0)
    return out.reshape(B, C, 1, 1).astype(np.float32)

